# revision 1
# baseline (speedup 1.0000x reference)
"""Trainium2 Bass kernel for nn_New3_77395310674432 (sparse_attention).

Pipeline (8-core SPMD, one NEFF):
  A) region = softmax(q@k.T/16) @ q, sharded by query rows (1250/core),
     computed as E^T = exp(k-chunk @ q-shard) then psum[m,257] += E^T.T @ [q|1].
  B) AllGather region (bf16, 5.1 MB).
  C) Per-core full item tables via factorized projections (no B*L*d3^2 work):
       tabK = feats@Wk.T   (384)      feats = [emb_item | region]
       tabG = feats@M, M = Wv.T@tc.T  (256; tc = this core's 256 gathered targets)
       tabB = feats@[Wq.T | Wc | bv]  (512; Wc folds the reshape-quirk key bias)
  D) Part-2 per batch shard (128/core): gather rows by user/item indices,
     s0 via the strided "reshape quirk" dot on DVE, u0 via diag-extract of
     gathered G rows, exp/mask/pow(beta=.5) -> predictions.
"""
import sys
if "/opt/trn_rl_repo" not in sys.path:
    sys.path.insert(0, "/opt/trn_rl_repo")
import numpy as np
import ml_dtypes

bf16 = ml_dtypes.bfloat16

N_ITEMS = 10000
D = 128
D2 = 256
D3 = 384
B = 1024
L = 100
NCORES = 8
NSH = N_ITEMS // NCORES      # 1250 items per core (stage A)
BSH = B // NCORES            # 128 batches per core
NPAD = 79 * 128              # 10112 padded items
NCH = 79                     # 128-row chunks
MBLOCKS = [(0, 512), (512, 512), (1024, 226)]  # stage-A m-blocks (per-core rows)
TABB_W = 512                 # [Q0 384 | Crow 100 | bvdot 1 | pad 27]
PEN = -1.0e9

_CACHE = {}


def _build_program(repeat=1, phases="ABCD"):
    import concourse.bass as bass
    import concourse.tile as tile
    from concourse import bacc, mybir
    from concourse.masks import make_identity

    F32 = mybir.dt.float32
    BF = mybir.dt.bfloat16
    I32 = mybir.dt.int32
    MUL = mybir.AluOpType.mult
    ADD = mybir.AluOpType.add

    nc = bacc.Bacc("TRN2", target_bir_lowering=False, debug=False,
                   num_devices=NCORES)

    def din(name, shape, dt):
        return nc.dram_tensor(name, shape, dt, kind="ExternalInput").ap()

    kt_d = din("kt", [2, 128, NPAD], BF)
    qt_d = din("qt", [2, 128, NSH], BF)
    qe_d = din("qe", [NCH, 128, D2 + 1], BF)
    embT_d = din("embT", [128, NPAD], BF)
    embg_d = din("embg", [N_ITEMS, D], BF)
    rhsK_d = din("rhsK", [3, 128, D3], BF)
    rhsB_d = din("rhsB", [3, 128, TABB_W], BF)
    wv3_d = din("wv3", [3, 128, D3], BF)
    consts_d = din("consts", [1, D3 + L], BF)
    user_d = din("user", [BSH, L], I32)
    item_d = din("item", [BSH, 2], I32)
    pred_d = nc.dram_tensor("pred", [BSH, 2], F32, kind="ExternalOutput").ap()

    with tile.TileContext(nc) as tc:
        with (
            tc.tile_pool(name="persist", bufs=1) as pp,
            tc.tile_pool(name="dram", bufs=1, space="DRAM") as dr,
        ):
            reg_sh = dr.tile([NSH, D2], BF)
            reg_full = dr.tile([N_ITEMS, D2], BF)
            tabKG = dr.tile([NPAD, D3 + D2], BF)
            tabB = dr.tile([NPAD, TABB_W], BF)

            # persistent small tiles
            ident = pp.tile([128, 128], BF)
            make_identity(nc, ident[:])
            user_t = pp.tile([BSH, L], I32)
            nc.sync.dma_start(user_t[:], user_d[:])
            item_t = pp.tile([BSH, 2], I32)
            nc.sync.dma_start(item_t[:], item_d[:])
            crow = pp.tile([1, D3 + L], BF)
            nc.sync.dma_start(crow[:], consts_d[:])
            crep = pp.tile([128, D3 + L], BF)
            nc.gpsimd.partition_broadcast(crep[:], crow[:])

            for rep in range(repeat):
                # ---------------- Phase A: region shard ----------------
                if "A" not in phases:
                    pass
                with (
                    tc.tile_pool(name=f"pa{rep}", bufs=1) as pa,
                    tc.tile_pool(name=f"pa_w{rep}", bufs=3) as pw,
                    tc.tile_pool(name=f"pa_ps{rep}", bufs=2, space="PSUM") as pps,
                    tc.tile_pool(name=f"pa_pr{rep}", bufs=1, space="PSUM") as ppr,
                ):
                    kt_sb = pa.tile([128, 2, NPAD], BF)
                    nc.sync.dma_start(kt_sb[:], kt_d[:].rearrange("c p n -> p c n"))
                    qt_sb = pa.tile([128, 2, NSH], BF)
                    nc.sync.dma_start(qt_sb[:], qt_d[:].rearrange("c p m -> p c m"))
                    qe_sb = pa.tile([128, NCH, D2 + 1], BF)
                    nc.sync.dma_start(qe_sb[:], qe_d[:].rearrange("c p w -> p c w"))

                    for m0, mbw in MBLOCKS:
                        nsub = (mbw + 127) // 128
                        psr = [ppr.tile([128, D2 + 1], F32, tag=f"psr{i}",
                                        name=f"psr{i}_{rep}")
                               for i in range(nsub)]
                        for ci in range(NCH):
                            psum_s = pps.tile([128, mbw], F32, tag="psum_s")
                            for kc in range(2):
                                nc.tensor.matmul(
                                    psum_s[:],
                                    kt_sb[:, kc, ci * 128:(ci + 1) * 128],
                                    qt_sb[:, kc, m0:m0 + mbw],
                                    start=(kc == 0), stop=(kc == 1))
                            e_sb = pw.tile([128, mbw], BF, tag="e_sb")
                            nc.scalar.activation(
                                e_sb[:], psum_s[:],
                                mybir.ActivationFunctionType.Exp, scale=1.0 / 16.0)
                            for si in range(nsub):
                                sw = min(128, mbw - si * 128)
                                nc.tensor.matmul(
                                    psr[si][:sw, :],
                                    e_sb[:, si * 128:si * 128 + sw],
                                    qe_sb[:, ci, :],
                                    start=(ci == 0), stop=(ci == NCH - 1))
                        for si in range(nsub):
                            r0 = m0 + si * 128
                            rows = min(128, NSH - r0)
                            rden = pw.tile([128, 1], F32, tag="rden")
                            nc.vector.reciprocal(rden[:rows], psr[si][:rows, D2:D2 + 1])
                            regmb = pw.tile([128, D2], BF, tag="regmb")
                            nc.vector.tensor_scalar_mul(
                                regmb[:rows], psr[si][:rows, 0:D2], rden[:rows])
                            nc.sync.dma_start(reg_sh[r0:r0 + rows, :], regmb[:rows])

                # ---------------- Phase B: AllGather region ----------------
                nc.gpsimd.collective_compute(
                    "AllGather", mybir.AluOpType.bypass,
                    replica_groups=[list(range(NCORES))],
                    ins=[reg_sh.opt()], outs=[reg_full.opt()])

                # ---------------- Phase C: tables ----------------
                with (
                    tc.tile_pool(name=f"pc{rep}", bufs=1) as pc,
                    tc.tile_pool(name=f"pc_w{rep}", bufs=3) as pcw,
                    tc.tile_pool(name=f"pc_ps{rep}", bufs=2, space="PSUM") as pcps,
                ):
                    et_sb = pc.tile([128, NPAD], BF)
                    nc.sync.dma_start(et_sb[:], embT_d[:])
                    rgT = pc.tile([128, 2, NPAD], BF)
                    nc.gpsimd.memset(rgT[:], 0.0)
                    for kc in range(2):
                        nc.sync.dma_start_transpose(
                            rgT[:, kc, 0:N_ITEMS],
                            reg_full[:, kc * 128:(kc + 1) * 128])
                    rk_sb = pc.tile([128, 3, D3], BF)
                    nc.sync.dma_start(rk_sb[:], rhsK_d[:].rearrange("c p w -> p c w"))
                    rb_sb = pc.tile([128, 3, TABB_W], BF)
                    nc.sync.dma_start(rb_sb[:], rhsB_d[:].rearrange("c p w -> p c w"))
                    wv_sb = pc.tile([128, 3, D3], BF)
                    nc.sync.dma_start(wv_sb[:], wv3_d[:].rearrange("c p w -> p c w"))

                    # targets: gather [emb | region] rows for item_i / item_j
                    tgt = pc.tile([128, 2, D3], BF)
                    for s in range(2):
                        nc.gpsimd.indirect_dma_start(
                            out=tgt[:, s, 0:D], out_offset=None, in_=embg_d[:],
                            in_offset=bass.IndirectOffsetOnAxis(
                                ap=item_t[:, s:s + 1], axis=0))
                        nc.gpsimd.indirect_dma_start(
                            out=tgt[:, s, D:D3], out_offset=None, in_=reg_full[:],
                            in_offset=bass.IndirectOffsetOnAxis(
                                ap=item_t[:, s:s + 1], axis=0))
                    # transpose targets -> tcT [feat, (pos128|neg128)]
                    tcT = pc.tile([128, 3, 2 * BSH], BF)
                    for oc in range(3):
                        for s in range(2):
                            pstr = pcps.tile([128, 128], BF, tag="pstr", bufs=1)
                            nc.tensor.transpose(
                                pstr[:], tgt[:, s, oc * 128:(oc + 1) * 128], ident[:])
                            nc.vector.tensor_copy(
                                tcT[:, oc, s * BSH:(s + 1) * BSH], pstr[:])
                    # M[in, tgt] = sum_out Wv[out, in] * tcT[out, tgt]
                    grhs = pc.tile([128, 3, 2 * BSH], BF)
                    for ic in range(3):
                        psM = pcps.tile([128, 2 * BSH], F32, tag="psM", bufs=1)
                        for oc in range(3):
                            nc.tensor.matmul(
                                psM[:], wv_sb[:, oc, ic * 128:(ic + 1) * 128],
                                tcT[:, oc, :], start=(oc == 0), stop=(oc == 2))
                        nc.vector.tensor_copy(grhs[:, ic, :], psM[:])

                    # table matmuls, 79 chunks of 128 items
                    for ch in range(NCH):
                        sl = slice(ch * 128, (ch + 1) * 128)
                        psK = pcps.tile([128, D3], F32, tag="psK")
                        psG = pcps.tile([128, D2], F32, tag="psG")
                        psB = pcps.tile([128, TABB_W], F32, tag="psB")
                        for j in range(3):
                            lh = et_sb[:, sl] if j == 0 else rgT[:, j - 1, sl]
                            nc.tensor.matmul(psK[:], lh, rk_sb[:, j, :],
                                             start=(j == 0), stop=(j == 2))
                            nc.tensor.matmul(psG[:], lh, grhs[:, j, :],
                                             start=(j == 0), stop=(j == 2))
                            nc.tensor.matmul(psB[:], lh, rb_sb[:, j, :],
                                             start=(j == 0), stop=(j == 2))
                        cKG = pcw.tile([128, D3 + D2], BF, tag="cKG")
                        nc.vector.tensor_copy(cKG[:, 0:D3], psK[:])
                        nc.scalar.copy(cKG[:, D3:D3 + D2], psG[:])
                        nc.sync.dma_start(tabKG[sl, :], cKG[:])
                        cB = pcw.tile([128, TABB_W], BF, tag="cB")
                        nc.vector.tensor_copy(cB[:], psB[:])
                        nc.sync.dma_start(tabB[sl, :], cB[:])

                # ---------------- Phase D: attention_network ----------------
                with (
                    tc.tile_pool(name=f"pd{rep}", bufs=1) as pd,
                    tc.tile_pool(name=f"pd_w{rep}", bufs=2) as pdw,
                ):
                    # gathers
                    ke = pd.tile([128, L, D3], BF)
                    bi = pd.tile([128, 2, TABB_W], BF)
                    for s in range(2):
                        nc.gpsimd.indirect_dma_start(
                            out=bi[:, s, :], out_offset=None, in_=tabB[:],
                            in_offset=bass.IndirectOffsetOnAxis(
                                ap=item_t[:, s:s + 1], axis=0))

                    preds = pd.tile([128, 2], F32)
                    ke_scr = ke[:].rearrange("p a b -> p (a b)").rearrange(
                        "p (d l) -> p l d", l=L)  # [128, L, D3] scrambled view

                    dens_all = pd.tile([128, 2], F32)
                    num_all = pd.tile([128, 2], F32)
                    bvd_all = pd.tile([128, 2], F32)

                    # merged K|G gather, l-chunked; diag-extract G in place
                    u0i = pd.tile([128, L, 2], F32)
                    LCH = 25
                    for l0 in range(0, L, LCH):
                        kg = pdw.tile([128, LCH, D3 + D2], BF, tag="kg", bufs=1)
                        for l in range(l0, l0 + LCH):
                            nc.gpsimd.indirect_dma_start(
                                out=kg[:, l - l0, :], out_offset=None,
                                in_=tabKG[:],
                                in_offset=bass.IndirectOffsetOnAxis(
                                    ap=user_t[:, l:l + 1], axis=0))
                        nc.scalar.copy(ke[:, l0:l0 + LCH, :], kg[:, :, 0:D3])
                        prod = pdw.tile([128, LCH, 2, BSH], BF, tag="prod")
                        nc.vector.tensor_tensor(
                            out=prod[:],
                            in0=kg[:, :, D3:D3 + D2].rearrange(
                                "p a (s t) -> p a s t", s=2),
                            in1=ident[:].unsqueeze(1).unsqueeze(1).to_broadcast(
                                [128, LCH, 2, BSH]),
                            op=MUL)
                        nc.vector.tensor_reduce(
                            u0i[:, l0:l0 + LCH, :].rearrange("p a b -> p (a b)"),
                            prod[:], axis=mybir.AxisListType.X, op=ADD)

                    for s in range(2):
                        qp = pdw.tile([128, D3], BF, tag="qp")
                        nc.vector.tensor_tensor(out=qp[:], in0=bi[:, s, 0:D3],
                                                in1=crep[:, 0:D3], op=ADD)
                        ct = pdw.tile([128, L], F32, tag="ct")
                        nc.vector.tensor_tensor(out=ct[:], in0=bi[:, s, D3:D3 + L],
                                                in1=crep[:, D3:D3 + L], op=ADD)
                        if s == 0:
                            eq = pdw.tile([128, L], F32, tag="eq")
                            nc.vector.tensor_tensor(
                                out=eq[:], in0=user_t[:],
                                in1=item_t[:, 0:1].to_broadcast([BSH, L]),
                                op=mybir.AluOpType.is_equal)
                            pen = pdw.tile([128, L], F32, tag="pen")
                            nc.vector.tensor_scalar_mul(pen[:], eq[:], PEN)
                            nc.vector.tensor_tensor(out=ct[:], in0=ct[:], in1=pen[:],
                                                    op=ADD)
                        # s0 via scrambled dot, l-chunked
                        s0 = pdw.tile([128, L], F32, tag="s0")
                        for l0 in range(0, L, LCH):
                            z = pdw.tile([128, LCH, D3], BF, tag="z")
                            nc.vector.tensor_tensor(
                                out=z[:], in0=ke_scr[:, l0:l0 + LCH, :],
                                in1=qp[:].unsqueeze(1).to_broadcast([128, LCH, D3]),
                                op=MUL)
                            nc.vector.tensor_reduce(
                                s0[:, l0:l0 + LCH], z[:],
                                axis=mybir.AxisListType.X, op=ADD)
                        nc.vector.tensor_tensor(out=s0[:], in0=s0[:], in1=ct[:], op=ADD)
                        expa = pdw.tile([128, L], F32, tag="expa")
                        den = pdw.tile([128, 1], F32, tag="den")
                        nc.scalar.activation(
                            expa[:], s0[:], mybir.ActivationFunctionType.Exp,
                            scale=float(1.0 / np.sqrt(D3)), accum_out=den[:])
                        nc.scalar.sqrt(dens_all[:, s:s + 1], den[:])
                        # num = sum_l expa * u0
                        wu = pdw.tile([128, L], F32, tag="wu")
                        nc.vector.tensor_tensor(out=wu[:], in0=expa[:],
                                                in1=u0i[:, :, s], op=MUL)
                        nc.vector.tensor_reduce(num_all[:, s:s + 1], wu[:],
                                                axis=mybir.AxisListType.X, op=ADD)
                        nc.vector.tensor_copy(bvd_all[:, s:s + 1],
                                              bi[:, s, D3 + L:D3 + L + 1])

                    # pred = num / dens + bvdot * dens
                    rdens = pd.tile([128, 2], F32)
                    nc.vector.reciprocal(rdens[:], dens_all[:])
                    t1 = pd.tile([128, 2], F32)
                    nc.vector.tensor_tensor(out=t1[:], in0=num_all[:], in1=rdens[:],
                                            op=MUL)
                    t2 = pd.tile([128, 2], F32)
                    nc.vector.tensor_tensor(out=t2[:], in0=bvd_all[:],
                                            in1=dens_all[:], op=MUL)
                    nc.vector.tensor_tensor(out=preds[:], in0=t1[:], in1=t2[:], op=ADD)
                    nc.sync.dma_start(pred_d[:], preds[:])

    nc.compile()
    return nc


def _prep_inputs(inputs):
    f = np.float32
    user = np.asarray(inputs["user"]).astype(np.int32)
    item_i = np.asarray(inputs["item_i"]).astype(np.int32)
    item_j = np.asarray(inputs["item_j"]).astype(np.int32)
    emb_item = np.asarray(inputs["emb_item"], dtype=f)
    emb_in = np.asarray(inputs["emb_in"], dtype=f)
    emb_out = np.asarray(inputs["emb_out"], dtype=f)
    Wq = np.asarray(inputs["Wq"], dtype=f)
    bq = np.asarray(inputs["bq"], dtype=f)
    Wk = np.asarray(inputs["Wk"], dtype=f)
    bk = np.asarray(inputs["bk"], dtype=f)
    Wv = np.asarray(inputs["Wv"], dtype=f)
    bv = np.asarray(inputs["bv"], dtype=f)

    q = np.concatenate([emb_in, emb_out], 1)            # [N, 256]
    k = np.concatenate([emb_out, emb_in], 1)
    kT = np.zeros((D2, NPAD), f)
    kT[:, :N_ITEMS] = k.T
    kt = kT.reshape(2, 128, NPAD).astype(bf16)
    qT = np.ascontiguousarray(q.T)                      # [256, 10000]
    qe = np.zeros((NPAD, D2 + 1), f)
    qe[:N_ITEMS, :D2] = q
    qe[:N_ITEMS, D2] = 1.0
    qe = qe.reshape(NCH, 128, D2 + 1).astype(bf16)
    embT = np.zeros((128, NPAD), f)
    embT[:, :N_ITEMS] = emb_item.T
    embT = embT.astype(bf16)
    embg = emb_item.astype(bf16)

    lgrid, dgrid = np.meshgrid(np.arange(L), np.arange(D3), indexing="ij")
    BKp = bk[(100 * dgrid + lgrid) % D3].astype(f)      # [L, D3]
    WqT = Wq.T
    Wc = WqT @ BKp.T                                    # [384, 100]
    cq = bq @ BKp.T                                     # [100]
    WkT = Wk.T
    rhsK = np.stack([WkT[128 * j:128 * (j + 1)] for j in range(3)]).astype(bf16)
    rhsB = np.zeros((3, 128, TABB_W), f)
    for j in range(3):
        rhsB[j, :, 0:D3] = WqT[128 * j:128 * (j + 1)]
        rhsB[j, :, D3:D3 + L] = Wc[128 * j:128 * (j + 1)]
        rhsB[j, :, D3 + L] = bv[128 * j:128 * (j + 1)]
    rhsB = rhsB.astype(bf16)
    wv3 = np.stack([Wv[128 * j:128 * (j + 1)] for j in range(3)]).astype(bf16)
    consts = np.zeros((1, D3 + L), f)
    consts[0, :D3] = bq
    consts[0, D3:] = cq
    consts = consts.astype(bf16)

    shared = dict(kt=kt, qe=qe, embT=embT, embg=embg, rhsK=rhsK, rhsB=rhsB,
                  wv3=wv3, consts=consts)
    in_maps = []
    for c in range(NCORES):
        qts = np.ascontiguousarray(
            qT[:, c * NSH:(c + 1) * NSH]).reshape(2, 128, NSH).astype(bf16)
        usr = user[c * BSH:(c + 1) * BSH]
        itm = np.stack([item_i[c * BSH:(c + 1) * BSH],
                        item_j[c * BSH:(c + 1) * BSH]], 1).astype(np.int32)
        m = dict(shared)
        m["qt"] = qts
        m["user"] = np.ascontiguousarray(usr)
        m["item"] = itm
        in_maps.append(m)
    return in_maps


def kernel(**inputs):
    from concourse.bass_utils import run_bass_kernel_spmd
    if "nc" not in _CACHE:
        _CACHE["nc"] = _build_program()
    nc = _CACHE["nc"]
    in_maps = _prep_inputs(inputs)
    res = run_bass_kernel_spmd(nc, in_maps, list(range(NCORES))).results
    pred_i = np.concatenate([res[c]["pred"][:, 0] for c in range(NCORES)])
    pred_j = np.concatenate([res[c]["pred"][:, 1] for c in range(NCORES)])
    return pred_i.astype(np.float32), pred_j.astype(np.float32)


if __name__ == "__main__":
    sys.path.insert(0, "/root/problem")
    import reference as R
    inp = R.setup_inputs()
    pi, pj = kernel(**{k: np.asarray(v) for k, v in inp.items()})
    ri, rj = R.reference(**inp)
    ri = np.asarray(ri); rj = np.asarray(rj)
    print("rel_i", np.max(np.abs(pi - ri)) / np.max(np.abs(ri)))
    print("rel_j", np.max(np.abs(pj - rj)) / np.max(np.abs(rj)))

